# revision 1
# baseline (speedup 1.0000x reference)
"""Trainium2 kernel for nn_GraphVToS_9388798509586 (gnn_message_passing).

Contract: kernel(**inputs) takes FULL unsharded numpy inputs and returns the
FULL [8, 128, 128, 64] float32 output.

Sharding: data-parallel over batch B=8 across the 8 NeuronCores (one batch
element per core), per the problem's sharding hint. Inside each core the
N x N pairwise grid is computed as two distance-weighted projections
(the concat-matmul split of the original GraphVToS layer):

    s[i,j,k] = sum_c d[i,j,c] * (p_i[i,c,k] + p_j[j,c,k]) + b[k]*sum_c d[i,j,c]
    out      = relu(s)

with p_i = vf @ w_vs[:F], p_j = vf @ w_vs[F:].

Execution goes through the PJRT path onto the 8 axon-tunneled trn2
NeuronCores via pmap (one program instance per core, batch-sharded inputs,
replicated weights). The gather back to the full output is a host-side
stack of the 8 per-core results.
"""

import numpy as np
import jax
import jax.numpy as jnp

# Problem shapes (hardcoded per contract; kernel.py must be self-contained).
B, N, C, F, K = 8, 128, 3, 64, 64

_N_CORES = 8


def _per_core(vf, d, w, b):
    """Compute one batch element's [N, N, K] output on one core.

    vf: [N, C, F] vector features for this batch element
    d:  [N, N, C] pairwise distance vectors
    w:  [2F, K] projection weights (replicated)
    b:  [K] bias (replicated)
    """
    w_i = w[:F]
    w_j = w[F:]
    # Per-atom projections: [N, C, K]
    p_i = jnp.einsum("ncf,fk->nck", vf, w_i)
    p_j = jnp.einsum("ncf,fk->nck", vf, w_j)
    # Distance-weighted reduction over the coordinate axis c.
    s = jnp.einsum("ick,ijc->ijk", p_i, d) + jnp.einsum("jck,ijc->ijk", p_j, d)
    s = s + d.sum(axis=-1)[..., None] * b
    return jax.nn.relu(s)


_pmapped = jax.pmap(_per_core, in_axes=(0, 0, None, None))
_jitted = jax.jit(jax.vmap(_per_core, in_axes=(0, 0, None, None)))


def kernel(**inputs: np.ndarray) -> np.ndarray:
    vf = np.ascontiguousarray(inputs["vector_features"], dtype=np.float32)
    d = np.ascontiguousarray(inputs["distances"], dtype=np.float32)
    w = np.ascontiguousarray(inputs["w_vs"], dtype=np.float32)
    b = np.ascontiguousarray(inputs["b_vs"], dtype=np.float32)

    if jax.device_count() >= _N_CORES and vf.shape[0] == _N_CORES:
        # One batch element per NeuronCore, weights replicated.
        out = _pmapped(vf, d, w, b)
    else:
        # Fallback for environments without the full 8-core mesh.
        out = _jitted(vf, d, w, b)
    return np.asarray(out, dtype=np.float32)


if __name__ == "__main__":
    rng = np.random.default_rng(0)
    ins = {
        "vector_features": rng.standard_normal((B, N, C, F), dtype=np.float32),
        "distances": rng.standard_normal((B, N, N, C), dtype=np.float32),
        "w_vs": rng.standard_normal((2 * F, K), dtype=np.float32),
        "b_vs": np.zeros((K,), dtype=np.float32),
    }
    out = kernel(**ins)
    print("out", out.shape, out.dtype, float(np.abs(out).max()))
